# revision 19
# baseline (speedup 1.0000x reference)
"""BandSplit kernel for Trainium2 (8 NeuronCores, batch-parallel).

Math (per band i with offset off, width b, K = 2b):
  x[t,k]   : band slice of X, k = re/im-interleaved bins (we reorder k = (c,f))
  z = ((x-mu)*rsqrt(var+eps)*gamma + beta) @ W + bias
    = rsqrt[t] * ( x @ Wg  +  mu[t]*(-colsum)  +  sigma[t]*cvec )
  with Wg = gamma*W (rows), colsum = sum_k Wg[k,:], cvec = beta@W + bias[i],
  sigma = sqrt(var+eps), rsqrt = 1/sigma.

So each output tile [128t, 512d] is ONE accumulation group of f32r matmuls
(lhsT = k-major x rows + a mu row + a sigma row, rhs = augmented W) followed
by a per-partition rsqrt scale fused into the PSUM->SBUF copy.

Per core: batch element b0 = core index. No collectives.
"""
import sys

sys.path.insert(0, "/opt/trn_rl_repo")
import numpy as np

BAND_BINS = [8] * 8 + [16] * 8 + [32] * 8 + [64] * 4 + [128] * 2 + [65]
NB = len(BAND_BINS)  # 31
D = 512
T = 1024
F = sum(BAND_BINS)  # 1025
EPS = 1e-5
NCORES = 8
NJ = T // 128  # 8 t-chunks


def plan():
    """Per-band chunk decomposition. Chunk rows: [x-rows ... , mu, sigma] where
    only the LAST chunk of a band carries the mu/sigma rows (x-rows in a chunk
    always start at tile partition 0). Returns list of bands:
      dict(off, b, chunks=[dict(rows_x, has_ms, wrow0)], ...)
    wrow0 = starting row of this chunk in the W_aug matrix."""
    bands = []
    off = 0
    wrow = 0
    for b in BAND_BINS:
        chunks = []
        if b >= 64:
            # split at the re/im boundary: both chunks compact without any
            # partition-shift DMA (chunk rows start at the c-plane start)
            chunks.append(dict(rows_x=b, has_ms=False, wrow0=wrow, xrow0=0))
            wrow += b
            if b + 2 <= 128:
                chunks.append(dict(rows_x=b, has_ms=True, wrow0=wrow, xrow0=b))
                wrow += b + 2
            else:
                chunks.append(dict(rows_x=b, has_ms=False, wrow0=wrow, xrow0=b))
                wrow += b
                chunks.append(dict(rows_x=0, has_ms=True, wrow0=wrow, xrow0=2 * b))
                wrow += 2
        else:
            chunks.append(dict(rows_x=2 * b, has_ms=True, wrow0=wrow, xrow0=0))
            wrow += 2 * b + 2
        bands.append(dict(off=off, b=b, chunks=chunks))
        off += b
    return bands, wrow


BANDS, W_ROWS = plan()  # W_ROWS == 2112


def build_w_aug(gamma, beta, W, bias):
    """Host-side: build the augmented, per-band-reordered weight matrix.
    k-order inside a band: r = c*b + f  (re plane rows then im plane rows)."""
    w_aug = np.zeros((W_ROWS, D), dtype=np.float32)
    wg = gamma[:, None] * W  # [2F, D]
    for i, bd in enumerate(BANDS):
        off, b = bd["off"], bd["b"]
        s2 = 2 * off
        # band row r -> reference k index
        kidx = np.empty(2 * b, dtype=np.int64)
        kidx[0:b] = s2 + 2 * np.arange(b)      # re rows (c=0)
        kidx[b:2 * b] = s2 + 2 * np.arange(b) + 1  # im rows (c=1)
        xw = wg[kidx]  # [2b, D]
        colsum = xw.sum(axis=0)
        cvec = beta[s2:s2 + 2 * b] @ W[s2:s2 + 2 * b] + bias[i]
        for ch in bd["chunks"]:
            r0, rx = ch["xrow0"], ch["rows_x"]
            w_aug[ch["wrow0"]:ch["wrow0"] + rx] = xw[r0:r0 + rx]
            if ch["has_ms"]:
                w_aug[ch["wrow0"] + rx] = -colsum
                w_aug[ch["wrow0"] + rx + 1] = cvec
    return w_aug


def build_nc():
    import concourse.bacc as bacc
    import concourse.tile as tile
    from concourse import mybir
    from concourse.masks import make_identity

    f32, f32r = mybir.dt.float32, mybir.dt.float32r
    nc = bacc.Bacc(None)
    X = nc.declare_dram_parameter("X", [F, T, 2], f32, isOutput=False)
    WA = nc.declare_dram_parameter("WA", [W_ROWS, D], f32r, isOutput=False)
    OUT = nc.declare_dram_parameter("OUT", [NB, T, D], f32, isOutput=True)

    Xf = X[:].rearrange("f t c -> f (t c)")  # [F, 2048]

    with tile.TileContext(nc) as tc:
        with tc.tile_pool(name="consts", bufs=1) as consts, \
             tc.tile_pool(name="nat", bufs=3) as natp, \
             tc.tile_pool(name="kx", bufs=6) as kxp, \
             tc.tile_pool(name="x2", bufs=3) as x2p, \
             tc.tile_pool(name="wp", bufs=12) as wp, \
             tc.tile_pool(name="tmp", bufs=3) as tmpp, \
             tc.tile_pool(name="stat", bufs=2) as statp, \
             tc.tile_pool(name="stage", bufs=2) as stagep, \
             tc.tile_pool(name="pso", bufs=4, space="PSUM") as psop, \
             tc.tile_pool(name="pss", bufs=2, space="PSUM") as pssp:

            ident = consts.tile([128, 128], f32)
            make_identity(nc, ident)
            ones_f = consts.tile([128, 2], f32)
            nc.vector.memset(ones_f, 1.0)
            ones = consts.tile([128, 2], f32r)
            nc.vector.tensor_scalar_mul(ones, ones_f, 1.0)
            epsc = consts.tile([128, 1], f32)
            nc.vector.memset(epsc, EPS)

            eng_flip = 0  # alternate DVE/ACT for balance

            for i, bd in enumerate(BANDS):
                off, b = bd["off"], bd["b"]
                inv_k = 1.0 / (2 * b)
                chunks = bd["chunks"]
                x_chunks = [c for c in chunks if c["rows_x"] > 0]
                last_x = len(x_chunks) - 1

                # --- load native band slice [b, 2048] (contiguous) ---
                nat = natp.tile([b, 2048], f32, tag="nat")
                nc.sync.dma_start(out=nat, in_=Xf[off:off + b, :])
                natv = nat[:, :].rearrange("f (t c) -> f c t", c=2)  # strided views

                # --- W chunks ---
                wts = []
                for ch in chunks:
                    rows = ch["rows_x"] + (2 if ch["has_ms"] else 0)
                    wt = wp.tile([rows, D], f32r, tag="W")
                    nc.sync.dma_start(out=wt, in_=WA[ch["wrow0"]:ch["wrow0"] + rows, :])
                    wts.append(wt)

                # --- compaction into k-major chunks ---
                kxs = []
                for ch in chunks:
                    rows = ch["rows_x"] + (2 if ch["has_ms"] else 0)
                    kx = kxp.tile([rows, T], f32r, tag="kx")
                    kxs.append(kx)
                # segments: band x-row r = c*b + f ; chunk-local row = r - xrow0
                for ci, ch in enumerate(chunks):
                    r0, rx = ch["xrow0"], ch["rows_x"]
                    if rx == 0:
                        continue
                    segs = []  # (c, f0, f1, dest_row0)
                    r = r0
                    while r < r0 + rx:
                        c, f = r // b, r % b
                        f1 = min(b, f + (r0 + rx - r))
                        segs.append((c, f, f1, r - r0))
                        r += f1 - f
                    for (c, f0, f1, dr) in segs:
                        n = f1 - f0
                        src = natv[f0:f1, c, :]
                        if dr == f0:
                            # same-base: direct strided compact+cast
                            if eng_flip % 2 == 0:
                                nc.vector.tensor_scalar_mul(kxs[ci][dr:dr + n, :], src, 1.0)
                            else:
                                nc.scalar.activation(
                                    out=kxs[ci][dr:dr + n, :], in_=src,
                                    func=mybir.ActivationFunctionType.Copy)
                            eng_flip += 1
                        else:
                            # cross-base: compact to same-base temp, DMA shift.
                            # engine APs need partition start 0 (or aligned
                            # power-of-2 blocks), so widen the op to start 0.
                            fa = 0
                            srcw = natv[fa:f1, c, :]
                            tmp = tmpp.tile([b, T], f32r, tag="imtmp")
                            if eng_flip % 2 == 0:
                                nc.vector.tensor_scalar_mul(tmp[fa:f1, :], srcw, 1.0)
                            else:
                                nc.scalar.activation(
                                    out=tmp[fa:f1, :], in_=srcw,
                                    func=mybir.ActivationFunctionType.Copy)
                            eng_flip += 1
                            nc.sync.dma_start(out=kxs[ci][dr:dr + n, :], in_=tmp[f0:f1, :])

                # --- stats: column sums via tiny N=2 matmuls ---
                # start=True clears has_written at BANK granularity, so all
                # matmuls of one accumulation group must be consecutive:
                # group (j) loops OUTER, k-chunks INNER.
                pc = pssp.tile([128, 32], f32, tag="pc")
                x2s = []
                for xi, ch in enumerate(x_chunks):
                    ci = chunks.index(ch)
                    rx = ch["rows_x"]
                    x2 = x2p.tile([rx, T], f32r, tag="x2")
                    nc.vector.tensor_mul(x2, kxs[ci][0:rx, :], kxs[ci][0:rx, :])
                    x2s.append(x2)
                for j in range(NJ):
                    for xi, ch in enumerate(x_chunks):
                        ci = chunks.index(ch)
                        rx = ch["rows_x"]
                        st, sp = (xi == 0), (xi == last_x)
                        nc.tensor.matmul(pc[:, 2 * j:2 * j + 2],
                                         kxs[ci][0:rx, j * 128:(j + 1) * 128],
                                         ones[0:rx, :], start=st, stop=sp)
                for j in range(NJ):
                    for xi, ch in enumerate(x_chunks):
                        rx = ch["rows_x"]
                        st, sp = (xi == 0), (xi == last_x)
                        nc.tensor.matmul(pc[:, 16 + 2 * j:18 + 2 * j],
                                         x2s[xi][:, j * 128:(j + 1) * 128],
                                         ones[0:rx, :], start=st, stop=sp)

                # --- per-band stats processing (batched over 8 t-chunks) ---
                ms = statp.tile([128, 16], f32, tag="ms")      # interleaved mu/sig cols
                msv = ms[:, :].rearrange("p (a c) -> p c a", c=2)
                rs = statp.tile([128, NJ], f32, tag="rs")      # rsqrt cols
                tmpe = statp.tile([128, NJ], f32, tag="tmpe")
                tmpm = statp.tile([128, NJ], f32, tag="tmpm")
                pcx = pc[:, 0:16].rearrange("p (a c) -> p c a", c=2)[:, 0, :]
                pcx2 = pc[:, 16:32].rearrange("p (a c) -> p c a", c=2)[:, 0, :]
                nc.vector.tensor_scalar_mul(msv[:, 0, :], pcx, inv_k)          # mu
                nc.vector.tensor_scalar_mul(tmpe, pcx2, inv_k)                 # E[x^2]
                nc.vector.tensor_mul(tmpm, msv[:, 0, :], msv[:, 0, :])         # mu^2
                nc.vector.tensor_sub(tmpe, tmpe, tmpm)                         # var
                nc.scalar.activation(out=msv[:, 1, :], in_=tmpe,
                                     func=mybir.ActivationFunctionType.Sqrt,
                                     bias=epsc, scale=1.0)                     # sigma
                nc.vector.reciprocal(out=rs, in_=msv[:, 1, :])                 # rsqrt

                # --- mu/sigma rows: PE transpose + partition-fold DMAs ---
                mt = pssp.tile([16, 128], f32, tag="mt")
                nc.tensor.transpose(mt, ms, ident)
                mts = statp.tile([16, 128], f32r, tag="mts")
                nc.vector.tensor_scalar_mul(mts, mt, 1.0)
                lc = chunks[-1]
                rem = lc["rows_x"]
                kxl = kxs[-1]
                nc.scalar.dma_start(
                    out=kxl[rem:rem + 1, :].rearrange("r (j p) -> r j p", j=NJ),
                    in_=mts[0:16:2, :])
                nc.scalar.dma_start(
                    out=kxl[rem + 1:rem + 2, :].rearrange("r (j p) -> r j p", j=NJ),
                    in_=mts[1:16:2, :])

                # --- main matmuls + fused rsqrt scale on PSUM->SBUF copy ---
                stage = stagep.tile([128, NJ, D], f32, tag="stage")
                for j in range(NJ):
                    po = psop.tile([128, D], f32, tag="po")
                    for ci, ch in enumerate(chunks):
                        rows = ch["rows_x"] + (2 if ch["has_ms"] else 0)
                        nc.tensor.matmul(po, kxs[ci][0:rows, j * 128:(j + 1) * 128],
                                         wts[ci][0:rows, :],
                                         start=(ci == 0), stop=(ci == len(chunks) - 1))
                    if (i + j) % 2 == 0:
                        nc.vector.tensor_scalar_mul(stage[:, j, :], po, rs[:, j:j + 1])
                    else:
                        nc.scalar.activation(out=stage[:, j, :], in_=po,
                                             func=mybir.ActivationFunctionType.Copy,
                                             scale=rs[:, j:j + 1])

                nc.sync.dma_start(
                    out=OUT[i].rearrange("(j p) d -> p j d", p=128), in_=stage)

    nc.finalize()
    return nc


_NC = None


def kernel(X, gamma, beta, W, bias):
    global _NC
    from concourse.bass_utils import run_bass_kernel_spmd

    X = np.asarray(X, dtype=np.float32)
    gamma = np.asarray(gamma, dtype=np.float32)
    beta = np.asarray(beta, dtype=np.float32)
    W = np.asarray(W, dtype=np.float32)
    bias = np.asarray(bias, dtype=np.float32)

    w_aug = build_w_aug(gamma, beta, W, bias)
    if _NC is None:
        _NC = build_nc()
    in_maps = [{"X": X[b], "WA": w_aug} for b in range(NCORES)]
    res = run_bass_kernel_spmd(_NC, in_maps, list(range(NCORES))).results
    return np.stack([res[b]["OUT"] for b in range(NCORES)], axis=0)
